# revision 20
# baseline (speedup 1.0000x reference)
"""Trainium2 Bass kernel for nn_BinaryTreeShInvariantConv.

Per (b, v): gather P=32 neighbor rows of signal[b] (Cin=64), contract over P
against conv_kernel[b,v] -> y[Cin, R*N], square, sum SH orders per degree l,
sqrt(+eps), contract [Cin*R*(L+1)=512] against kernel_weights -> [Cout=128],
bias + relu.

Sharding: data-parallel over batch B=8 -> one batch per NeuronCore (SPMD).

Dataflow per core (one batch, V=4096), in supergroups of 128 v's:
  - dma_gather: 4096 rows (128 v x 32 p) of bf16-padded signal -> patches
    [128 part=(j4,p32), 32 chunk, 64 ch].  Gather idx table is wrapped into
    16 partitions and replicated x8 (the ucode reads [128, 256]-shaped APs
    but only consumes the first 16-partition stripe; sub-128-partition APs
    are illegal off bases 0/32/64/96, and unwritten SBUF cannot be read, so
    the x8 replication is the cheapest legal layout); one dma, loaded once.
  - K: pair-block-diagonal layout (matmul operands may only start at
    partition 0/32/64, so a full 32-deep per-j contraction is impossible):
    kbd[kg] = [128=(j,p), (sgin4, j'2, g32, rn32)], where row 32j+p has
    K[v=4g+j] in its j'==j%2 block and zeros in the other (memset once at
    startup; diagonals refilled per 512-v group by 4 dmas -> 32 dmas total).
  - MM1 per chunk g, per pair pr: lhsT = pat[64pr:64pr+64, g, :] (64-deep
    contraction at base 0/64), rhs = kbd slice [64, (j'2, rn32)] ->
    psum[64h+c, 128blk+64pr+32j'+rn]  (h=g%2, blk=(g%16)//2, bank=g//16).
  - square (ACT) -> ysq bf16; degree-sum (DVE reduce over n-windows
    1,3,5,7) -> zpre f32; sqrt(x+eps) (ACT) -> zsb bf16.
  - MM3 flipped: lhsT = W [64 c, 128 i], rhs = zsb slice [64 c, 64 (a,v)],
    accumulate 8 rl-slices -> psum [128 i, 128 (h,a,v)]  (half the PE rows
    of the unflipped order).
  - relu (DVE max, or ACT bias+relu) -> osb [128 i, 256] (2 supergroups),
    one dma per 2 sg to outd [128 i, 4096 v-cols]; host untransposes.
"""

import sys

sys.path.insert(0, "/opt/trn_rl_repo")

import numpy as np

import concourse.bacc as bacc
import concourse.mybir as mybir
import concourse.tile as tile
from concourse import bass2jax

B, V, P, CIN, R, COUT = 8, 4096, 32, 64, 2, 128
NSH, NDEG = 16, 4
VSG = 128            # v's per supergroup
NSG = V // VSG       # 32 supergroups
NCHUNK = VSG // 4    # 32 chunks of 4 v's
SGI = VSG * P        # 4096 gather indices per supergroup
KG = 4               # compute-supergroups per K-load group
NKG = NSG // KG      # 8 K loads
BF16 = mybir.dt.bfloat16
F32 = mybir.dt.float32
I16 = mybir.dt.int16

IDXC = (SGI // 16) * NSG  # idx tile columns: 256 per supergroup

_CACHE = {}
_SKIP = set()  # debug: subset of {'gather','mm1','post','mm3','store'}


def _dma_gather_any(eng, out_ap, in_ap, idxs_ap, num_idxs, elem_size,
                    single_packet=True):
    """bass.dma_gather minus the elem_size%256 assert (the Q7 ucode only
    requires the source ROW STRIDE to be a 256B multiple; the bytes read per
    row are free). in_ap's outer stride (elem_step) must be 256B-aligned."""
    from concourse import ap_utils
    from concourse.bass import MemorySpace

    assert idxs_ap.dtype == I16
    assert in_ap.space == MemorySpace.DRAM
    assert in_ap.dtype == out_ap.dtype
    elem_step = in_ap.ap[0][0]
    stride_bytes = elem_step * mybir.dt.size(in_ap.dtype)
    assert stride_bytes % 256 == 0 and stride_bytes // 256 < 256
    assert ap_utils.ap_is_contiguous(out_ap.ap[1:])
    assert ap_utils.ap_is_contiguous(idxs_ap.ap[1:])
    assert in_ap.ap[-1][1] == out_ap.ap[-1][1] == elem_size
    assert out_ap.ap[0][1] * out_ap.ap[1][1] == ((num_idxs + 127) // 128) * 128

    _in_ap = eng.lower_ap_dma(in_ap, for_custom_bir_dma=True)
    return eng.add_instruction(
        mybir.InstDMAGatherAnt(
            name=eng.bass.get_next_instruction_name(),
            ins=[*_in_ap, eng.lower_ap(idxs_ap),
                 eng.lower_val_access(eng.to_reg(num_idxs))],
            outs=[eng.lower_ap(out_ap)],
            transpose=False,
            num_idxs=num_idxs,
            elem_size=elem_size,
            stride_bytes_256=stride_bytes // 256,
            gen_mode=0,
            single_packet=single_packet,
            queue_num=0,
            sbuf_tokens_per_rank=0,
            sbuf_free_dim_per_rank=0,
            sbuf_free_dim_pad_per_rank=0,
            sbuf_byte_offset=0,
        ))


def _build_nc(nsg, with_bias):
    nc = bacc.Bacc("TRN2", target_bir_lowering=False, debug=False,
                   enable_asserts=False)
    vtot = nsg * VSG
    nkg = nsg // KG
    sig = nc.dram_tensor("sig", [V, 128], BF16, kind="ExternalInput")
    kre = nc.dram_tensor("kre", [nkg, 128, KG * NCHUNK * R * NSH], BF16,
                         kind="ExternalInput")
    kcols = KG * 2 * NCHUNK * R * NSH  # 8192: (sgin4, j'2, g32, rn32)
    idx = nc.dram_tensor("idx", [128, IDXC], I16, kind="ExternalInput")
    wsb = nc.dram_tensor("wsb", [128, 8 * COUT], BF16, kind="ExternalInput")
    bia = nc.dram_tensor("bia", [1, COUT], F32, kind="ExternalInput")
    # [i, v]-major bf16 output; host untransposes/unpermutes/upcasts
    outd = nc.dram_tensor("outd", [COUT, vtot], BF16, kind="ExternalOutput")

    AF = mybir.ActivationFunctionType
    with tile.TileContext(nc) as tc:
        with (
            tc.tile_pool(name="const", bufs=1) as constp,
            tc.tile_pool(name="kbd", bufs=2) as kbdp,
            tc.tile_pool(name="patches", bufs=3) as patp,
            tc.tile_pool(name="ysq", bufs=3) as ysqp,
            tc.tile_pool(name="zpre", bufs=3) as zprep,
            tc.tile_pool(name="zsb", bufs=3) as zsbp,
            tc.tile_pool(name="osb", bufs=3) as osbp,
            tc.tile_pool(name="ps1", bufs=3, space="PSUM") as ps1p,
            tc.tile_pool(name="ps3", bufs=2, space="PSUM") as ps3p,
        ):
            w_t = constp.tile([128, 8 * COUT], BF16, tag="w")
            nc.sync.dma_start(w_t[:], wsb.ap())
            # idx loaded in two pieces: sg0-3's columns first (0.7us) so the
            # first gather isn't gated on the full 2MB table; the rest is
            # issued from inside the loop (sg==1), overlapping the pipeline.
            idx_t = constp.tile([128, IDXC], I16, tag="idx")
            idxc0 = 4 * (SGI // 16)
            nc.sync.dma_start(idx_t[:, 0:idxc0], idx.ap()[:, 0:idxc0])
            if with_bias:
                bias_t = constp.tile([128, 1], F32, tag="bias")
                nc.sync.dma_start(bias_t[:], bia.ap().rearrange("o i -> i o"))

            eps_t = constp.tile([128, 1], F32, tag="eps")
            nc.vector.memset(eps_t[:], 1e-4)

            kbds = [kbdp.tile([128, kcols], BF16, tag="kbd",
                              name=f"kbd{i}") for i in range(2)]
            nc.vector.memset(kbds[0][:], 0.0)
            # kbd1 zeroed on gpsimd, issued inside the sg==1 iteration so it
            # queues AFTER gather0's desc-gen on the Pool engine (it is only
            # needed by kg=1's MM1, much later).

            osb = None
            for sg in range(nsg):
                kg, sgin = sg // KG, sg % KG
                # --- K diagonal refill: 4 dmas per 512-v group ----------
                kbd = kbds[kg % 2]
                kbd_r = kbd[:, :].rearrange("p (s j g r) -> p s j g r",
                                            s=KG, j=2, g=NCHUNK)
                if sgin == 0:
                    for j in range(4):
                        nc.sync.dma_start(
                            kbd_r[32 * j:32 * (j + 1), :, j % 2],
                            kre.ap()[kg, 32 * j:32 * (j + 1), :].rearrange(
                                "p (s g r) -> p s g r", s=KG, g=NCHUNK))

                if sg == 1:
                    # zero kbd1 after gather0's desc-gen in Pool queue order
                    nc.gpsimd.memset(kbds[1][:], 0.0)
                    nc.sync.dma_start(idx_t[:, idxc0:], idx.ap()[:, idxc0:])

                # --- gather patches -------------------------------------
                pat = patp.tile([128, NCHUNK, CIN], BF16, tag="pat")
                if 'gather' not in _SKIP:
                    _dma_gather_any(
                        nc.gpsimd, pat[:, :, :], sig.ap()[:, 0:CIN],
                        idx_t[:, (SGI // 16) * sg:(SGI // 16) * (sg + 1)],
                        SGI, CIN, single_packet=False)

                # --- MM1: per (chunk, pair) conv over p -----------------
                ps1 = [ps1p.tile([128, 1024], F32, tag="ps1",
                                 name=f"ps1_{q}") for q in range(2)]
                for g in range(NCHUNK if 'mm1' not in _SKIP else 0):
                    bank, blk, h = g // 16, (g % 16) // 2, g % 2
                    for pr in range(2):
                        lhsT = pat[64 * pr:64 * (pr + 1), g, :]
                        rhs = kbd_r[64 * pr:64 * (pr + 1), sgin, :, g, :]
                        out = ps1[bank][64 * h:64 * (h + 1),
                                        128 * blk + 64 * pr:
                                        128 * blk + 64 * (pr + 1)]
                        nc.tensor.matmul(out, lhsT, rhs, start=True,
                                         stop=True)

                # --- square on ACT --------------------------------------
                ysq = ysqp.tile([128, 2048], BF16, tag="ysq")
                for q in range(2 if 'post' not in _SKIP else 0):
                    dst = ysq[:, 1024 * q:1024 * (q + 1)]
                    nc.scalar.activation(dst, ps1[q][:], AF.Square)

                # --- degree sums over n-windows -------------------------
                zpre = zprep.tile([128, 512], F32, tag="zpre")
                ysq_r = ysq[:, :].rearrange("p (a b n) -> p a b n", a=16, b=8)
                zpre_r = zpre[:, :].rearrange("p (a b l) -> p a b l", a=16,
                                              b=8)
                for l in range(NDEG if 'post' not in _SKIP else 0):
                    w_l = 2 * l + 1
                    nc.vector.reduce_sum(
                        zpre_r[:, :, :, l],
                        ysq_r[:, :, :, l * l:l * l + w_l],
                        axis=mybir.AxisListType.X)

                # --- sqrt(x + eps) --------------------------------------
                zsb = zsbp.tile([128, 512], BF16, tag="zsb")
                if 'post' not in _SKIP:
                    nc.scalar.activation(zsb[:], zpre[:], AF.Sqrt,
                                         bias=eps_t[:])

                # --- MM3 (flipped): out[i, (h,a,v)] ---------------------
                ps3 = ps3p.tile([128, VSG], F32, tag="ps3")
                zsb_r = zsb[:, :].rearrange("p (a v rl) -> p a v rl", a=16,
                                            v=4)
                for h in range(2 if 'mm3' not in _SKIP else 0):
                    pa, pb = 64 * h, 64 * (h + 1)
                    for rl in range(8):
                        lhsT = w_t[pa:pb, COUT * rl:COUT * (rl + 1)]
                        rhs = zsb_r[pa:pb, :, :, rl]
                        nc.tensor.matmul(ps3[:, pa:pb], lhsT, rhs,
                                         start=(rl == 0), stop=(rl == 7),
                                         skip_group_check=True)

                # --- bias + relu; store every 2 sg ----------------------
                if sg % 2 == 0:
                    osb = osbp.tile([128, 2 * VSG], BF16, tag="osb")
                dst = osb[:, VSG * (sg % 2):VSG * (sg % 2 + 1)]
                if with_bias:
                    nc.scalar.activation(dst, ps3[:], AF.Relu,
                                         bias=bias_t[:])
                else:
                    nc.vector.tensor_scalar_max(dst, ps3[:], 0.0)
                if sg % 2 == 1 and 'store' not in _SKIP:
                    nc.sync.dma_start(
                        outd.ap()[:, VSG * (sg - 1):VSG * (sg + 1)],
                        osb[:, :])

    nc.compile()
    return nc


def _prep_inputs_core(b, signal, patches_idx, conv_kernel, kernel_weights,
                      biases, nsg):
    bf = mybir.dt.np(BF16)
    sig = np.zeros((V, 128), dtype=bf)
    sig[:, :CIN] = signal[b].astype(bf)
    # kre[kg, 32j+p, (sgin, g, rn)] = K[b, 512kg+128sgin+4g+j, p, rn]
    k = conv_kernel[b].reshape(nsg // KG, KG, NCHUNK, 4, P, R * NSH)
    kre = np.ascontiguousarray(
        k.transpose(0, 3, 4, 1, 2, 5)).reshape(nsg // KG, 128, -1).astype(bf)
    # wsb[c + 64*dup, rl*128 + i] = kernel_weights[i, c, r, l]
    w = kernel_weights.transpose(2, 3, 1, 0).reshape(8, CIN, COUT)
    wrow = np.ascontiguousarray(w.transpose(1, 0, 2)).reshape(CIN, 8 * COUT)
    wsb = np.concatenate([wrow, wrow], axis=0).astype(bf)
    bia = biases.reshape(1, COUT).astype(np.float32)
    idx = _fix_idx_wrap(patches_idx[b, :, :, 1].astype(np.int16).reshape(-1))
    return {"sig": sig, "kre": kre, "idx": idx, "wsb": wsb, "bia": bia}


def _fix_idx_wrap(pidx_flat):
    """Wrap per-sg gather indices into 16 partitions (the stripe the SWDGE
    ucode consumes: gather position i of op sg comes from partition i%16,
    column 256*sg + i//16), then replicate x8 to fill the [128, *] AP the
    ucode requires."""
    pidx = pidx_flat.reshape(NSG, VSG, P)  # [sg, v_in_sg, p]
    i = np.arange(SGI)
    vi = 4 * (i // 128) + (i % 128) // 32
    pi = (i % 128) % 32
    buf = np.zeros((16, IDXC), dtype=np.int16)
    for sg in range(NSG):
        buf[i % 16, (SGI // 16) * sg + i // 16] = pidx[sg, vi, pi]
    return np.tile(buf, (8, 1))


# column permutation of outd: out[b, v, i] = outd_b[i, _OUT_COL[v]]
def _out_colmap():
    v = np.arange(V)
    q = (v % 128) // 64
    blk = (v % 64) // 8
    h = (v % 8) // 4
    j = v % 4
    a = 8 * q + blk
    return 128 * (v // 128) + 64 * h + 4 * a + j


_OUT_COL = _out_colmap()


def _make_runner(nc, n_cores=8):
    import jax
    from jax.sharding import Mesh, PartitionSpec
    from jax.experimental.shard_map import shard_map

    bass2jax.install_neuronx_cc_hook()
    partition_name = (nc.partition_id_tensor.name
                      if nc.partition_id_tensor else None)
    in_names, out_names, out_avals, zero_outs = [], [], [], []
    for alloc in nc.m.functions[0].allocations:
        if not isinstance(alloc, mybir.MemoryLocationSet):
            continue
        name = alloc.memorylocations[0].name
        if alloc.kind == "ExternalInput":
            if name != partition_name:
                in_names.append(name)
        elif alloc.kind == "ExternalOutput":
            out_names.append(name)
            shape = tuple(alloc.tensor_shape)
            dtype = mybir.dt.np(alloc.dtype)
            out_avals.append(jax.core.ShapedArray(shape, dtype))
            zero_outs.append(np.zeros(shape, dtype))
    n_params, n_outs = len(in_names), len(out_avals)
    in_names_all = list(in_names) + list(out_names)
    if partition_name is not None:
        in_names_all.append(partition_name)

    def _body(*args):
        operands = list(args)
        if partition_name is not None:
            operands.append(bass2jax.partition_id_tensor())
        outs = bass2jax._bass_exec_p.bind(
            *operands, out_avals=tuple(out_avals),
            in_names=tuple(in_names_all), out_names=tuple(out_names),
            lowering_input_output_aliases=(),
            sim_require_finite=True, sim_require_nnan=True, nc=nc)
        return tuple(outs)

    donate = tuple(range(n_params, n_params + n_outs))
    devices = jax.devices()[:n_cores]
    mesh = Mesh(np.asarray(devices), ("core",))
    sharded = jax.jit(
        shard_map(_body, mesh=mesh,
                  in_specs=(PartitionSpec("core"),) * (n_params + n_outs),
                  out_specs=(PartitionSpec("core"),) * n_outs,
                  check_rep=False),
        donate_argnums=donate, keep_unused=True)

    def run_fn(in_maps):
        import jax
        per_core = [[np.asarray(m[nm]) for nm in in_names] for m in in_maps]
        concat_in = [
            np.concatenate([per_core[c][i] for c in range(n_cores)], axis=0)
            for i in range(n_params)]
        concat_zeros = [
            np.zeros((n_cores * z.shape[0], *z.shape[1:]), z.dtype)
            for z in zero_outs]
        out_arrs = sharded(*concat_in, *concat_zeros)
        jax.block_until_ready(out_arrs)
        return [
            {nm: np.asarray(out_arrs[i]).reshape(n_cores, *out_avals[i].shape)[c]
             for i, nm in enumerate(out_names)}
            for c in range(n_cores)]

    return run_fn


def kernel(signal, patches_idx, conv_kernel, kernel_weights, biases):
    with_bias = bool(np.any(biases))
    key = ("k", NSG, with_bias)
    if key not in _CACHE:
        nc = _build_nc(NSG, with_bias)
        _CACHE[key] = (nc, _make_runner(nc))
    nc, run = _CACHE[key]

    in_maps = []
    for b in range(B):
        m = _prep_inputs_core(b, signal, patches_idx, conv_kernel,
                              kernel_weights, biases, NSG)
        in_maps.append(m)

    results = run(in_maps)
    # outd is [COUT, V] with permuted v columns; untranspose on host
    out = np.stack([results[b]["outd"][:, _OUT_COL].T for b in range(B)],
                   axis=0)
    return out.astype(np.float32)
